# revision 31
# baseline (speedup 1.0000x reference)
"""CRF NLL loss kernel for Trainium2 (8 NeuronCores, data-parallel over batch).

Math: the forward recurrence alpha_t = LSE_j(alpha_{t-1,j} + trans[j,k]) + emit_t
is computed in probability space:  P_t = Eemit_t * (Etrans^T @ P_{t-1})
with per-step normalizers d_t = mean_b LSE_k(emit[t,b,:]) (host-precomputed).

Parallel-segment decomposition: products of positive matrices contract in the
Hilbert projective metric (diagonal emission scalings are isometries), so a
chain started from an arbitrary positive vector converges to the true state's
DIRECTION within ~16 steps; magnitudes differ by one scalar per column which
the host recovers by stitching shipped boundary states.  This turns the
T-step serial recurrence into 4 concurrent streams, all elementwise on DVE
(homogeneous streams avoid the in-order-queue resonance that mixed engine
classes lock into; 4 streams saturate the DVE at ~633ns/step throughput):
  forward:  F1 (t 1..136), F2 (121..255, 16-step burn-in)
  backward: B1 (t 510..375), B2 (391..256, burn)
Backward runs in X-space X_t = Ehat_t * (M @ X_{t+1}); variable sequence ends
are emission-rewrites on the host: padded steps hold the Perron vector r of
M = exp(trans) (scalar 1/lambda per step) and the single boundary step uses
v/r, v = M^{-1} exp(etrans), mapping r -> w exactly.
Host computes logZ_b = log(P(255) . M X(256)) + stitch scalars + D[end_b];
the gold-path score is pure gather work, done on host in f64.
"""

import numpy as np
import ml_dtypes

import concourse.bacc as bacc
import concourse.mybir as mybir
import concourse.tile as tile
from concourse.bass_utils import run_bass_kernel_spmd

T, B, N = 512, 256, 128
NCORES = 8
BL = B // NCORES          # 32 sequences per core
K = 16                    # burn-in steps for non-initial streams
M1 = 136                  # forward boundary
N1 = 375                  # backward boundary
CHUNK = 32                # emission steps per DMA chunk
F0 = 8                    # first-chunk steps folded into init DMA (F1/B1)

BF = ml_dtypes.bfloat16

# 4 homogeneous DVE streams at the DVE-throughput period (4 x 158.3 ~ 633):
# a uniform engine class avoids the in-order-queue resonance that mixed
# 551/884 chain classes lock into.
def _mk_streams():
    f1 = list(range(1, M1 + 1))
    f2 = list(range(M1 - K + 1, 256))
    b1 = list(range(510, N1 - 1, -1))
    b2 = list(range(N1 + K, 255, -1))
    return [
        dict(name="F1", eng="dve", dr="F", ts=f1, ships={len(f1) - 1: 0},
             init="dma", first=F0, period=633, phase=0),
        dict(name="F2", eng="dve", dr="F", ts=f2,
             ships={K - 1: 1, len(f2) - 1: 2}, init="ones", first=K,
             period=633, phase=158),
        dict(name="B1", eng="dve", dr="B", ts=b1, ships={len(b1) - 1: 3},
             init="dma", first=F0, period=633, phase=316),
        dict(name="B2", eng="dve", dr="B", ts=b2,
             ships={K: 4, len(b2) - 1: 5}, init="ones", first=K,
             period=633, phase=474),
    ]

STREAMS = _mk_streams()
NSHIP = 6

# Optional {(stream, k): vtime} emission-order override (self-consistent
# schedule measured from a TimelineSim run; see EMIT_ORDER below).
SCHEDULE = None

LAST_RESULTS = None       # BassKernelResults of the last run (for profiling)

_compiled = {}


def _build_nc():
    nc = bacc.Bacc("TRN2", target_bir_lowering=False, debug=False,
                   num_devices=NCORES)
    f32 = mybir.dt.float32
    bf16 = mybir.dt.bfloat16

    # merged critical loads: [state0 | weights | first chunks of same-direction
    # streams] -> one HWDGE generation slot each (the generator is shared
    # across queues and serializes at ~630ns)
    IWF = BL + N + F0 * BL + K * BL
    initf = nc.dram_tensor("initf", [N, IWF], bf16, kind="ExternalInput")
    initb = nc.dram_tensor("initb", [N, IWF], bf16, kind="ExternalInput")
    emd = {}
    for st in STREAMS:
        w = (len(st["ts"]) - st["first"]) * BL
        emd[st["name"]] = nc.dram_tensor("em" + st["name"], [N, max(w, BL)],
                                         bf16, kind="ExternalInput")
    ships = nc.dram_tensor("ships", [N, NSHIP * BL], bf16,
                           kind="ExternalOutput")

    with tile.TileContext(nc) as tc:
        with (
            tc.tile_pool(name="const", bufs=1) as cpool,
            tc.tile_pool(name="emit", bufs=max(
                (len(st["ts"]) - st["first"] + CHUNK - 1) // CHUNK
                for st in STREAMS)) as epool,
            tc.tile_pool(name="state", bufs=3) as spool,
            tc.tile_pool(name="tmp", bufs=3) as tpool,
            tc.tile_pool(name="ps", bufs=1, space="PSUM") as pspool,
        ):
            tF0 = cpool.tile([N, IWF], bf16, tag="initf")
            nc.sync.dma_start(tF0[:], initf[:])
            tB0 = cpool.tile([N, IWF], bf16, tag="initb")
            nc.gpsimd.dma_start(tB0[:], initb[:])

            mF = tF0[:, BL:BL + N]
            mB = tB0[:, BL:BL + N]

            # ones init for burn-in streams
            t_ones = cpool.tile([N, BL], bf16, tag="ones")
            nc.vector.memset(t_ones[:], 1.0)

            # per-stream runtime state
        # layout of first-chunk region inside init tiles:
            #   [state0 | M | first(F1 or B1) | first(F2/B2) | first(F3/B3)]
            offs = {
                "F1": BL + N, "B1": BL + N,
                "F2": BL + N + F0 * BL, "B2": BL + N + F0 * BL,
            }
            rt = {}
            chunk_reqs = []   # (need_vtime, stream, chunk lo, hi)
            for st in STREAMS:
                nm = st["name"]
                it = tF0 if st["dr"] == "F" else tB0
                n_steps = len(st["ts"])
                # emission AP per step (first chunk lives in the init tile)
                em_ap = []
                for k in range(st["first"]):
                    em_ap.append((it, offs[nm] + k * BL))
                n_rest = n_steps - st["first"]
                n_ch = (n_rest + CHUNK - 1) // CHUNK
                for c in range(n_ch):
                    lo = c * CHUNK * BL
                    hi = min(n_rest, (c + 1) * CHUNK) * BL
                    need = st["phase"] + (st["first"] + c * CHUNK) * st["period"]
                    chunk_reqs.append((need, nm, lo, hi))
                cur = it[:, 0:BL] if st["init"] == "dma" else t_ones[:]
                rt[nm] = dict(st=st, cur=cur, em=em_ap,
                              m=mF if st["dr"] == "F" else mB)

            # all chunk DMAs on SP, ordered by first-consumption time, so the
            # Activation sequencer serves only the hybrid-stream copies
            chunk_reqs.sort()
            for _, nm, lo, hi in chunk_reqs:
                ch_t = epool.tile([N, CHUNK * BL], bf16, tag="em" + nm)
                nc.sync.dma_start(ch_t[:, :hi - lo], emd[nm][:, lo:hi])
                for k in range((hi - lo) // BL):
                    rt[nm]["em"].append((ch_t, k * BL))

            # ship destination tiles (dedicated, never recycled)
            ship_t = []
            for i in range(NSHIP):
                sh = cpool.tile([N, BL], bf16, tag=f"ship{i}")
                ship_t.append(sh)

            # static virtual-time schedule
            events = []
            for st in STREAMS:
                for k in range(len(st["ts"])):
                    vt = None
                    if SCHEDULE is not None:
                        vt = SCHEDULE.get((st["name"], k))
                    if vt is None:
                        vt = st["phase"] + k * st["period"]
                    events.append((vt, st["name"], k))
            events.sort()

            for _, nm, k in events:
                r = rt[nm]
                st = r["st"]
                ps_t = pspool.tile([N, BL], f32, tag="ps" + nm)
                nc.tensor.matmul(ps_t[:], r["m"], r["cur"],
                                 start=True, stop=True)
                ch_t, off = r["em"][k]
                em = ch_t[:, off:off + BL]
                slot = st["ships"].get(k)
                if slot is None:
                    o_t = spool.tile([N, BL], bf16, tag="s" + nm)
                    out = o_t[:]
                else:
                    out = ship_t[slot][:]
                if st["eng"] == "dve":
                    nc.vector.tensor_tensor(out, ps_t[:], em,
                                            mybir.AluOpType.mult)
                else:
                    tm_t = tpool.tile([N, BL], bf16, tag="t" + nm)
                    nc.scalar.copy(tm_t[:], ps_t[:])
                    nc.gpsimd.tensor_tensor(out, tm_t[:], em,
                                            mybir.AluOpType.mult)
                r["cur"] = out
                if slot is not None:
                    q = nc.sync if st["dr"] == "F" else nc.scalar
                    q.dma_start(ships[:, slot * BL:(slot + 1) * BL], out)
    nc.compile()
    return nc


def kernel(emit, target, mask, trans, strans, etrans):
    global LAST_RESULTS
    emit = np.asarray(emit, dtype=np.float32)
    target = np.asarray(target, dtype=np.int32)
    mask = np.asarray(mask)
    trans = np.asarray(trans, dtype=np.float32)
    strans = np.asarray(strans, dtype=np.float32)
    etrans = np.asarray(etrans, dtype=np.float32)

    # --- host preprocessing ---
    e64 = emit.astype(np.float64)
    m_t = e64.max(axis=2, keepdims=True)
    lse = (m_t[..., 0] + np.log(np.exp(e64 - m_t).sum(axis=2)))  # [T,B]
    d = lse.mean(axis=1)
    d[0] = 0.0
    D = np.cumsum(d)

    eemn = np.exp(e64 - d[:, None, None])                        # [T,B,N]
    M64 = np.exp(trans.astype(np.float64))                       # [N,N] (j,k)
    w64 = np.exp(etrans.astype(np.float64))

    r = np.ones(N, dtype=np.float64)
    for _ in range(60):
        r = M64 @ r
        r /= r.mean()
    lam = float((r @ (M64 @ r)) / (r @ r))
    v = np.linalg.solve(M64, w64)

    L = mask.astype(np.int64).sum(axis=0)
    ends = L - 1

    P0 = np.exp(strans[None, :].astype(np.float64) + e64[0]).T   # [N,B]

    # backward emissions Ehat_t for t=256..510 indexed [N, t, B]
    tt = np.arange(256, 511)
    EB = eemn[256:511].transpose(2, 0, 1).copy()                 # [N,255,B]
    pad = (tt[None, :] > L[:, None]).T[None, :, :]
    bnd = (tt[None, :] == L[:, None]).T[None, :, :]
    EB = np.where(pad, 1.0 / lam, EB)
    EB = np.where(bnd, (v / r)[:, None, None], EB)

    def em_at(t):
        # [N, B] emission consumed at step t (fwd E'_t or bwd Ehat_t)
        if t <= 255:
            return eemn[t].T
        return EB[:, t - 256, :]

    X0 = np.empty((N, B), dtype=np.float64)
    full = L == T
    last = L == T - 1
    rest = ~(full | last)
    if full.any():
        X0[:, full] = (eemn[511, full, :] * w64[None, :]).T
    if last.any():
        X0[:, last] = v[:, None]
    if rest.any():
        X0[:, rest] = (r / lam)[:, None]

    # per-stream emission arrays in consumption order
    em_all = {}
    for st in STREAMS:
        em_all[st["name"]] = np.stack([em_at(t) for t in st["ts"]],
                                      axis=1)                    # [N,steps,B]

    in_maps = []
    Mbf = M64.astype(BF)
    MTbf = np.ascontiguousarray(M64.T).astype(BF)
    for c in range(NCORES):
        sl = slice(c * BL, (c + 1) * BL)
        im = {}
        for drn, s0, mm, first_sts in (
                ("initf", P0[:, sl], Mbf, ("F1", "F2")),
                ("initb", X0[:, sl], MTbf, ("B1", "B2"))):
            parts = [s0.astype(BF), mm]
            for nm in first_sts:
                st = next(s for s in STREAMS if s["name"] == nm)
                parts.append(em_all[nm][:, :st["first"], sl]
                             .reshape(N, -1).astype(BF))
            im[drn] = np.ascontiguousarray(np.concatenate(parts, axis=1))
        for st in STREAMS:
            nm = st["name"]
            rest_a = em_all[nm][:, st["first"]:, sl].reshape(N, -1)
            if rest_a.shape[1] == 0:
                rest_a = np.zeros((N, BL))
            im["em" + nm] = np.ascontiguousarray(rest_a.astype(BF))
        in_maps.append(im)

    if "nc" not in _compiled:
        _compiled["nc"] = _build_nc()
    nc = _compiled["nc"]

    res = run_bass_kernel_spmd(nc, in_maps, core_ids=list(range(NCORES)))
    LAST_RESULTS = res

    # --- host postprocessing: stitch shipped boundary states ---
    sh = np.concatenate(
        [res.results[c]["ships"].astype(np.float64) for c in range(NCORES)]
        , axis=0).reshape(NCORES, N, NSHIP * BL)
    S = [np.concatenate([sh[c][:, i * BL:(i + 1) * BL]
                         for c in range(NCORES)], axis=1)
         for i in range(NSHIP)]                                  # each [N,B]
    f1b, f2a, f2b, b1b, b2a, b2b = S

    def ratio(a, b):
        return (a * b).sum(axis=0) / (b * b).sum(axis=0)

    s2 = ratio(f1b, f2a)                                         # [B]
    u2 = ratio(b1b, b2a)                                         # [B]
    Q = M64 @ b2b                                                # [N,B]
    dot = (f2b * Q).sum(axis=0)
    logZ = (np.log(dot) + np.log(s2) + np.log(u2) + D[ends]).sum()

    # gold score (f64, mirrors reference)
    tb = np.arange(B)
    emit_sc = np.take_along_axis(e64, target[:, :, None].astype(np.int64),
                                 axis=2)[..., 0]
    trans_sc = trans.astype(np.float64)[target[:-1], target[1:]]
    scores = emit_sc.copy()
    scores[1:] += trans_sc
    score = np.where(mask, scores, 0.0).sum()
    score += strans.astype(np.float64)[target[0]].sum()
    score += etrans.astype(np.float64)[target[ends, tb]].sum()

    loss = (logZ - score) / B
    return np.float32(loss)


# revision 32
# speedup vs baseline: 1.4007x; 1.4007x over previous
"""CRF NLL loss kernel for Trainium2 (8 NeuronCores, data-parallel over batch).

Math: the forward recurrence alpha_t = LSE_j(alpha_{t-1,j} + trans[j,k]) + emit_t
is computed in probability space:  P_t = Eemit_t * (Etrans^T @ P_{t-1})
with per-step normalizers d_t = mean_b LSE_k(emit[t,b,:]) (host-precomputed).

Parallel-segment decomposition: products of positive matrices contract in the
Hilbert projective metric (diagonal emission scalings are isometries), so a
chain started from an arbitrary positive vector converges to the true state's
DIRECTION within ~16 steps; magnitudes differ by one scalar per column which
the host recovers by stitching shipped boundary states.  The T-step serial
recurrence becomes 6 concurrent segments, run as 3 PAIRS (one forward + one
backward segment each).  A pair advances with two matmuls into disjoint
halves of one PSUM tile plus ONE combined [128,64] DVE multiply - the DVE's
per-instruction PSUM-access charge (125ns) is paid once per pair instead of
once per stream, so 3 pairs fit under the ~643ns chain latency:
  pair1: F1 (t 1..96, true init) + B1 (t 510..415, true init)
  pair2: F2 (t 81..176, 16-step burn-in) + B2 (t 431..336, burn)
  pair3: F3 (t 159..255, 18-step burn)   + B3 (t 352..256, burn)
Backward runs in X-space X_t = Ehat_t * (M @ X_{t+1}); variable sequence ends
are emission-rewrites on the host: padded steps hold the Perron vector r of
M = exp(trans) (scalar 1/lambda per step) and the single boundary step uses
v/r, v = M^{-1} exp(etrans), mapping r -> w exactly.
Host computes logZ_b = log(P(255) . M X(256)) + stitch scalars + D[end_b];
the gold-path score is pure gather work, done on host in f64.
"""

import numpy as np
import ml_dtypes

import concourse.bacc as bacc
import concourse.mybir as mybir
import concourse.tile as tile
from concourse.bass_utils import run_bass_kernel_spmd

T, B, N = 512, 256, 128
NCORES = 8
BL = B // NCORES          # 32 sequences per core
W2 = 2 * BL               # paired tile width
CHUNK = 16                # emission steps per DMA chunk (2*BL wide each)

BF = ml_dtypes.bfloat16

# Each pair: equal-length F and B segments advancing in lockstep.
# ships: {step_idx: slot} per member; slot order fixed for host stitching:
#  0 f1b  1 f2a  2 f2b  3 f3a  4 f3b  5 b1b  6 b2a  7 b2b  8 b3a  9 b3b
def _mk_pairs():
    return [
        dict(name="P1", first=8, phase=0,
             F=dict(ts=list(range(1, 97)), init="dma", ships={95: 0}),
             Bk=dict(ts=list(range(510, 414, -1)), init="dma", ships={95: 5})),
        dict(name="P2", first=16, phase=214,
             F=dict(ts=list(range(81, 177)), init="ones",
                    ships={15: 1, 95: 2}),
             Bk=dict(ts=list(range(431, 335, -1)), init="ones",
                     ships={16: 6, 95: 7})),
        dict(name="P3", first=18, phase=428,
             F=dict(ts=list(range(159, 256)), init="ones",
                    ships={17: 3, 96: 4}),
             Bk=dict(ts=list(range(352, 255, -1)), init="ones",
                     ships={16: 8, 96: 9})),
    ]

PAIRS = _mk_pairs()
NSHIP = 10
PERIOD = 643

LAST_RESULTS = None       # BassKernelResults of the last run (for profiling)

_compiled = {}


def _build_nc():
    nc = bacc.Bacc("TRN2", target_bir_lowering=False, debug=False,
                   num_devices=NCORES)
    f32 = mybir.dt.float32
    bf16 = mybir.dt.bfloat16

    # init0 packs [pair1 state0 (F|B) | mF | mB | pair1 first chunk] so the
    # critical lead-in is one HWDGE generation slot; pair2/3 first chunks ride
    # a second DMA on the SWDGE queue (their phases start later anyway).
    IW0 = W2 + 2 * N + PAIRS[0]["first"] * W2
    IW1 = PAIRS[1]["first"] * W2 + PAIRS[2]["first"] * W2
    init0 = nc.dram_tensor("init0", [N, IW0], bf16, kind="ExternalInput")
    init1 = nc.dram_tensor("init1", [N, IW1], bf16, kind="ExternalInput")
    emd = {}
    for p in PAIRS:
        w = (len(p["F"]["ts"]) - p["first"]) * W2
        emd[p["name"]] = nc.dram_tensor("em" + p["name"], [N, max(w, W2)],
                                        bf16, kind="ExternalInput")
    ships = nc.dram_tensor("ships", [N, NSHIP * BL], bf16,
                           kind="ExternalOutput")

    with tile.TileContext(nc) as tc:
        with (
            tc.tile_pool(name="const", bufs=1) as cpool,
            tc.tile_pool(name="emit", bufs=max(
                (len(p["F"]["ts"]) - p["first"] + CHUNK - 1) // CHUNK
                for p in PAIRS)) as epool,
            tc.tile_pool(name="state", bufs=3) as spool,
            tc.tile_pool(name="ps", bufs=1, space="PSUM") as pspool,
        ):
            t00 = cpool.tile([N, IW0], bf16, tag="init0")
            nc.sync.dma_start(t00[:], init0[:])
            t01 = cpool.tile([N, IW1], bf16, tag="init1")
            nc.gpsimd.dma_start(t01[:], init1[:])

            mF = t00[:, W2:W2 + N]
            mB = t00[:, W2 + N:W2 + 2 * N]

            t_ones = cpool.tile([N, W2], bf16, tag="ones")
            nc.vector.memset(t_ones[:], 1.0)

            # first-chunk base offsets
            fbase = {"P1": (t00, W2 + 2 * N), "P2": (t01, 0),
                     "P3": (t01, PAIRS[1]["first"] * W2)}

            rt = {}
            chunk_reqs = []
            for p in PAIRS:
                nm = p["name"]
                n_steps = len(p["F"]["ts"])
                em_ap = []
                it, base = fbase[nm]
                for k in range(p["first"]):
                    em_ap.append((it, base + k * W2))
                n_rest = n_steps - p["first"]
                for c in range((n_rest + CHUNK - 1) // CHUNK):
                    lo = c * CHUNK * W2
                    hi = min(n_rest, (c + 1) * CHUNK) * W2
                    need = p["phase"] + (p["first"] + c * CHUNK) * PERIOD
                    chunk_reqs.append((need, nm, lo, hi))
                cur = (t00[:, 0:W2] if p["F"]["init"] == "dma"
                       else t_ones[:])
                rt[nm] = dict(p=p, cur=cur, em=em_ap)

            chunk_reqs.sort()
            for _, nm, lo, hi in chunk_reqs:
                ch_t = epool.tile([N, CHUNK * W2], bf16, tag="em" + nm)
                nc.sync.dma_start(ch_t[:, :hi - lo], emd[nm][:, lo:hi])
                for k in range((hi - lo) // W2):
                    rt[nm]["em"].append((ch_t, k * W2))

            ship_t = {}   # (pair, k) -> dedicated combined out tile

            events = []
            for p in PAIRS:
                for k in range(len(p["F"]["ts"])):
                    events.append((p["phase"] + k * PERIOD, p["name"], k))
            events.sort()

            for _, nm, k in events:
                r = rt[nm]
                p = r["p"]
                ps_t = pspool.tile([N, W2], f32, tag="ps" + nm)
                cur = r["cur"]
                nc.tensor.matmul(ps_t[:, 0:BL], mF, cur[:, 0:BL],
                                 start=True, stop=True)
                nc.tensor.matmul(ps_t[:, BL:W2], mB, cur[:, BL:W2],
                                 start=True, stop=True)
                ch_t, off = r["em"][k]
                shf = p["F"]["ships"].get(k)
                shb = p["Bk"]["ships"].get(k)
                if shf is not None or shb is not None:
                    o_t = cpool.tile([N, W2], bf16, tag=f"sh{nm}_{k}")
                else:
                    o_t = spool.tile([N, W2], bf16, tag="s" + nm)
                nc.vector.tensor_tensor(o_t[:], ps_t[:],
                                        ch_t[:, off:off + W2],
                                        mybir.AluOpType.mult)
                r["cur"] = o_t
                if shf is not None:
                    nc.sync.dma_start(ships[:, shf * BL:(shf + 1) * BL],
                                      o_t[:, 0:BL])
                if shb is not None:
                    nc.scalar.dma_start(ships[:, shb * BL:(shb + 1) * BL],
                                        o_t[:, BL:W2])
    nc.compile()
    return nc


def kernel(emit, target, mask, trans, strans, etrans):
    global LAST_RESULTS
    emit = np.asarray(emit, dtype=np.float32)
    target = np.asarray(target, dtype=np.int32)
    mask = np.asarray(mask)
    trans = np.asarray(trans, dtype=np.float32)
    strans = np.asarray(strans, dtype=np.float32)
    etrans = np.asarray(etrans, dtype=np.float32)

    # --- host preprocessing ---
    e64 = emit.astype(np.float64)
    m_t = e64.max(axis=2, keepdims=True)
    lse = (m_t[..., 0] + np.log(np.exp(e64 - m_t).sum(axis=2)))  # [T,B]
    d = lse.mean(axis=1)
    d[0] = 0.0
    D = np.cumsum(d)

    eemn = np.exp(e64 - d[:, None, None])                        # [T,B,N]
    M64 = np.exp(trans.astype(np.float64))                       # [N,N] (j,k)
    w64 = np.exp(etrans.astype(np.float64))

    r = np.ones(N, dtype=np.float64)
    for _ in range(60):
        r = M64 @ r
        r /= r.mean()
    lam = float((r @ (M64 @ r)) / (r @ r))
    v = np.linalg.solve(M64, w64)

    L = mask.astype(np.int64).sum(axis=0)
    ends = L - 1

    P0 = np.exp(strans[None, :].astype(np.float64) + e64[0]).T   # [N,B]

    tt = np.arange(256, 511)
    EB = eemn[256:511].transpose(2, 0, 1).copy()                 # [N,255,B]
    pad = (tt[None, :] > L[:, None]).T[None, :, :]
    bnd = (tt[None, :] == L[:, None]).T[None, :, :]
    EB = np.where(pad, 1.0 / lam, EB)
    EB = np.where(bnd, (v / r)[:, None, None], EB)

    def em_at(t):
        if t <= 255:
            return eemn[t].T
        return EB[:, t - 256, :]

    X0 = np.empty((N, B), dtype=np.float64)
    full = L == T
    last = L == T - 1
    rest = ~(full | last)
    if full.any():
        X0[:, full] = (eemn[511, full, :] * w64[None, :]).T
    if last.any():
        X0[:, last] = v[:, None]
    if rest.any():
        X0[:, rest] = (r / lam)[:, None]

    # per-pair interleaved emission arrays [N, steps, 2, B]
    em_all = {}
    for p in PAIRS:
        ef = np.stack([em_at(t) for t in p["F"]["ts"]], axis=1)  # [N,s,B]
        eb = np.stack([em_at(t) for t in p["Bk"]["ts"]], axis=1)
        em_all[p["name"]] = np.stack([ef, eb], axis=2)           # [N,s,2,B]

    in_maps = []
    Mbf = M64.astype(BF)
    MTbf = np.ascontiguousarray(M64.T).astype(BF)
    for c in range(NCORES):
        sl = slice(c * BL, (c + 1) * BL)
        im = {}
        pe = {nm: em_all[nm][:, :, :, sl].reshape(N, -1).astype(BF)
              for nm in em_all}
        f0 = PAIRS[0]["first"] * W2
        im["init0"] = np.ascontiguousarray(np.concatenate(
            [P0[:, sl].astype(BF), X0[:, sl].astype(BF), Mbf, MTbf,
             pe["P1"][:, :f0]], axis=1))
        im["init1"] = np.ascontiguousarray(np.concatenate(
            [pe["P2"][:, :PAIRS[1]["first"] * W2],
             pe["P3"][:, :PAIRS[2]["first"] * W2]], axis=1))
        for p in PAIRS:
            nm = p["name"]
            rest_a = pe[nm][:, p["first"] * W2:]
            if rest_a.shape[1] == 0:
                rest_a = np.zeros((N, W2), dtype=BF)
            im["em" + nm] = np.ascontiguousarray(rest_a)
        in_maps.append(im)

    if "nc" not in _compiled:
        _compiled["nc"] = _build_nc()
    nc = _compiled["nc"]

    res = run_bass_kernel_spmd(nc, in_maps, core_ids=list(range(NCORES)))
    LAST_RESULTS = res

    # --- host postprocessing: stitch shipped boundary states ---
    S = [np.concatenate(
        [res.results[c]["ships"][:, i * BL:(i + 1) * BL].astype(np.float64)
         for c in range(NCORES)], axis=1) for i in range(NSHIP)]
    f1b, f2a, f2b, f3a, f3b, b1b, b2a, b2b, b3a, b3b = S

    def ratio(a, b):
        return (a * b).sum(axis=0) / (b * b).sum(axis=0)

    s3 = ratio(f1b, f2a) * ratio(f2b, f3a)                       # [B]
    u3 = ratio(b1b, b2a) * ratio(b2b, b3a)                       # [B]
    Q = M64 @ b3b                                                # [N,B]
    dot = (f3b * Q).sum(axis=0)
    logZ = (np.log(dot) + np.log(s3) + np.log(u3) + D[ends]).sum()

    # gold score (f64, mirrors reference)
    tb = np.arange(B)
    emit_sc = np.take_along_axis(e64, target[:, :, None].astype(np.int64),
                                 axis=2)[..., 0]
    trans_sc = trans.astype(np.float64)[target[:-1], target[1:]]
    scores = emit_sc.copy()
    scores[1:] += trans_sc
    score = np.where(mask, scores, 0.0).sum()
    score += strans.astype(np.float64)[target[0]].sum()
    score += etrans.astype(np.float64)[target[ends, tb]].sum()

    loss = (logZ - score) / B
    return np.float32(loss)


# revision 34
# speedup vs baseline: 1.4765x; 1.0541x over previous
"""CRF NLL loss kernel for Trainium2 (8 NeuronCores, data-parallel over batch).

Math: the forward recurrence alpha_t = LSE_j(alpha_{t-1,j} + trans[j,k]) + emit_t
is computed in probability space:  P_t = Eemit_t * (Etrans^T @ P_{t-1})
with per-step normalizers d_t = mean_b LSE_k(emit[t,b,:]) (host-precomputed).

Parallel-segment decomposition: products of positive matrices contract in the
Hilbert projective metric (diagonal emission scalings are isometries), so a
chain started from an arbitrary positive vector converges to the true state's
DIRECTION within ~16 steps; magnitudes differ by one scalar per column which
the host recovers by stitching shipped boundary states.  The T-step serial
recurrence becomes 9 concurrent segments (5 forward + 4 backward), run as 3
TRIPLES of 76 lockstep steps each.  A triple advances with three matmuls into
thirds of one PSUM tile plus ONE combined [128,96] DVE multiply - the DVE's
per-instruction PSUM-access charge (125ns) is paid once per triple, fitting
3 triples under the ~680ns chain latency.  Interior segments use burn-ins of
15-31 steps (every spare step extends a burn-in, improving convergence).
Backward runs in X-space X_t = Ehat_t * (M @ X_{t+1}); variable sequence ends
are emission-rewrites on the host: padded steps hold the Perron vector r of
M = exp(trans) (scalar 1/lambda per step) and the single boundary step uses
v/r, v = M^{-1} exp(etrans), mapping r -> w exactly.
Host computes logZ_b = log(P(255) . M X(256)) + stitch scalars + D[end_b];
the gold-path score is pure gather work, done on host in f64.
"""

import numpy as np
import ml_dtypes

import concourse.bacc as bacc
import concourse.mybir as mybir
import concourse.tile as tile
from concourse.bass_utils import run_bass_kernel_spmd

T, B, N = 512, 256, 128
NCORES = 8
BL = B // NCORES          # 32 sequences per core
W3 = 3 * BL               # triple tile width
STEPS = 76                # lockstep steps per triple
FIRST = 16                # emission steps folded into init DMAs
CHUNK = 15                # emission steps per DMA chunk (W3 wide)
PERIOD = 680

BF = ml_dtypes.bfloat16

# members: (dir, consumed t-list, init, ships {step_idx: slot})
# slots: even = segment end state, odd = post-burn-in state; stitch pairs:
# fwd (0,1)(2,3)(4,5)(6,7) final 8; bwd (9,10)(11,12)(13,14) final 15.
def _mk_triples():
    F, Bk = "F", "B"
    return [
        dict(name="T1", phase=0, members=[
            (F, list(range(1, 77)), "dma", {75: 0}),
            (F, list(range(46, 122)), "ones", {30: 1, 75: 2}),
            (F, list(range(91, 167)), "ones", {30: 3, 75: 4}),
        ]),
        dict(name="T2", phase=226, members=[
            (F, list(range(136, 212)), "ones", {30: 5, 75: 6}),
            (F, list(range(180, 256)), "ones", {31: 7, 75: 8}),
            (Bk, list(range(510, 434, -1)), "dma", {75: 9}),
        ]),
        dict(name="T3", phase=452, members=[
            (Bk, list(range(450, 374, -1)), "ones", {15: 10, 75: 11}),
            (Bk, list(range(390, 314, -1)), "ones", {15: 12, 75: 13}),
            (Bk, list(range(331, 255, -1)), "ones", {16: 14, 75: 15}),
        ]),
    ]

TRIPLES = _mk_triples()
NSHIP = 16

LAST_RESULTS = None       # BassKernelResults of the last run (for profiling)

_compiled = {}


def _build_nc():
    nc = bacc.Bacc("TRN2", target_bir_lowering=False, debug=False,
                   num_devices=NCORES)
    f32 = mybir.dt.float32
    bf16 = mybir.dt.bfloat16

    # init0 (HWDGE slot 1): [P0 | X0 | mF | mB | T1 first chunk]
    # init1 (SWDGE queue): [T2 first chunk | T3 first chunk]
    IW0 = 2 * BL + 2 * N + FIRST * W3
    IW1 = 2 * FIRST * W3
    init0 = nc.dram_tensor("init0", [N, IW0], bf16, kind="ExternalInput")
    init1 = nc.dram_tensor("init1", [N, IW1], bf16, kind="ExternalInput")
    emd = {}
    for tr in TRIPLES:
        w = (STEPS - FIRST) * W3
        emd[tr["name"]] = nc.dram_tensor("em" + tr["name"], [N, w], bf16,
                                         kind="ExternalInput")
    ships = nc.dram_tensor("ships", [N, NSHIP * BL], bf16,
                           kind="ExternalOutput")

    with tile.TileContext(nc) as tc:
        with (
            tc.tile_pool(name="const", bufs=1) as cpool,
            tc.tile_pool(name="emit", bufs=(STEPS - FIRST + CHUNK - 1)
                         // CHUNK) as epool,
            tc.tile_pool(name="state", bufs=3) as spool,
            tc.tile_pool(name="ps", bufs=1, space="PSUM") as pspool,
        ):
            t00 = cpool.tile([N, IW0], bf16, tag="init0")
            nc.sync.dma_start(t00[:], init0[:])
            t01 = cpool.tile([N, IW1], bf16, tag="init1")
            nc.gpsimd.dma_start(t01[:], init1[:])

            mF = t00[:, 2 * BL:2 * BL + N]
            mB = t00[:, 2 * BL + N:2 * BL + 2 * N]

            t_ones = cpool.tile([N, BL], bf16, tag="ones")
            nc.vector.memset(t_ones[:], 1.0)

            fbase = {"T1": (t00, 2 * BL + 2 * N), "T2": (t01, 0),
                     "T3": (t01, FIRST * W3)}

            rt = {}
            chunk_reqs = []
            for tr in TRIPLES:
                nm = tr["name"]
                em_ap = []
                it, base = fbase[nm]
                for k in range(FIRST):
                    em_ap.append((it, base + k * W3))
                for c in range((STEPS - FIRST + CHUNK - 1) // CHUNK):
                    lo = c * CHUNK * W3
                    hi = min(STEPS - FIRST, (c + 1) * CHUNK) * W3
                    need = tr["phase"] + (FIRST + c * CHUNK) * PERIOD
                    chunk_reqs.append((need, nm, lo, hi))
                # per-member current-state APs
                curs = []
                for mi, (dr, ts, init, sh) in enumerate(tr["members"]):
                    if init == "dma":
                        curs.append(t00[:, 0:BL] if dr == "F"
                                    else t00[:, BL:2 * BL])
                    else:
                        curs.append(t_ones[:])
                rt[nm] = dict(tr=tr, curs=curs, em=em_ap)

            chunk_reqs.sort()
            for _, nm, lo, hi in chunk_reqs:
                ch_t = epool.tile([N, CHUNK * W3], bf16, tag="em" + nm)
                nc.sync.dma_start(ch_t[:, :hi - lo], emd[nm][:, lo:hi])
                for k in range((hi - lo) // W3):
                    rt[nm]["em"].append((ch_t, k * W3))

            events = []
            for tr in TRIPLES:
                for k in range(STEPS):
                    events.append((tr["phase"] + k * PERIOD, tr["name"], k))
            events.sort()

            for _, nm, k in events:
                r = rt[nm]
                tr = r["tr"]
                ps_t = pspool.tile([N, W3], f32, tag="ps" + nm)
                for mi, (dr, ts, init, sh) in enumerate(tr["members"]):
                    nc.tensor.matmul(ps_t[:, mi * BL:(mi + 1) * BL],
                                     mF if dr == "F" else mB, r["curs"][mi],
                                     start=True, stop=True)
                ch_t, off = r["em"][k]
                shd = [sh.get(k) for (_, _, _, sh) in tr["members"]]
                if any(s is not None for s in shd):
                    o_t = cpool.tile([N, W3], bf16, tag=f"sh{nm}_{k}")
                else:
                    o_t = spool.tile([N, W3], bf16, tag="s" + nm)
                nc.vector.tensor_tensor(o_t[:], ps_t[:],
                                        ch_t[:, off:off + W3],
                                        mybir.AluOpType.mult)
                for mi in range(3):
                    r["curs"][mi] = o_t[:, mi * BL:(mi + 1) * BL]
                    if shd[mi] is not None:
                        q = nc.sync if mi % 2 == 0 else nc.scalar
                        q.dma_start(
                            ships[:, shd[mi] * BL:(shd[mi] + 1) * BL],
                            o_t[:, mi * BL:(mi + 1) * BL])
    nc.compile()
    return nc


def kernel(emit, target, mask, trans, strans, etrans):
    global LAST_RESULTS
    emit = np.asarray(emit, dtype=np.float32)
    target = np.asarray(target, dtype=np.int32)
    mask = np.asarray(mask)
    trans = np.asarray(trans, dtype=np.float32)
    strans = np.asarray(strans, dtype=np.float32)
    etrans = np.asarray(etrans, dtype=np.float32)

    # --- host preprocessing ---
    e64 = emit.astype(np.float64)
    m_t = e64.max(axis=2, keepdims=True)
    lse = (m_t[..., 0] + np.log(np.exp(e64 - m_t).sum(axis=2)))  # [T,B]
    d = lse.mean(axis=1)
    d[0] = 0.0
    D = np.cumsum(d)

    eemn = np.exp(e64 - d[:, None, None])                        # [T,B,N]
    M64 = np.exp(trans.astype(np.float64))                       # [N,N] (j,k)
    w64 = np.exp(etrans.astype(np.float64))

    r = np.ones(N, dtype=np.float64)
    for _ in range(60):
        r = M64 @ r
        r /= r.mean()
    lam = float((r @ (M64 @ r)) / (r @ r))
    v = np.linalg.solve(M64, w64)

    L = mask.astype(np.int64).sum(axis=0)
    ends = L - 1

    P0 = np.exp(strans[None, :].astype(np.float64) + e64[0]).T   # [N,B]

    tt = np.arange(256, 511)
    EB = eemn[256:511].transpose(2, 0, 1).copy()                 # [N,255,B]
    pad = (tt[None, :] > L[:, None]).T[None, :, :]
    bnd = (tt[None, :] == L[:, None]).T[None, :, :]
    EB = np.where(pad, 1.0 / lam, EB)
    EB = np.where(bnd, (v / r)[:, None, None], EB)

    def em_at(t):
        if t <= 255:
            return eemn[t].T
        return EB[:, t - 256, :]

    X0 = np.empty((N, B), dtype=np.float64)
    full = L == T
    last = L == T - 1
    rest = ~(full | last)
    if full.any():
        X0[:, full] = (eemn[511, full, :] * w64[None, :]).T
    if last.any():
        X0[:, last] = v[:, None]
    if rest.any():
        X0[:, rest] = (r / lam)[:, None]

    # per-triple interleaved emissions [N, steps, 3, B]
    em_all = {}
    for tr in TRIPLES:
        ems = [np.stack([em_at(t) for t in ts], axis=1)
               for (_, ts, _, _) in tr["members"]]
        em_all[tr["name"]] = np.stack(ems, axis=2)               # [N,s,3,B]

    in_maps = []
    Mbf = M64.astype(BF)
    MTbf = np.ascontiguousarray(M64.T).astype(BF)
    for c in range(NCORES):
        sl = slice(c * BL, (c + 1) * BL)
        pe = {nm: em_all[nm][:, :, :, sl].reshape(N, -1).astype(BF)
              for nm in em_all}
        f0 = FIRST * W3
        im = {
            "init0": np.ascontiguousarray(np.concatenate(
                [P0[:, sl].astype(BF), X0[:, sl].astype(BF), Mbf, MTbf,
                 pe["T1"][:, :f0]], axis=1)),
            "init1": np.ascontiguousarray(np.concatenate(
                [pe["T2"][:, :f0], pe["T3"][:, :f0]], axis=1)),
        }
        for nm in pe:
            im["em" + nm] = np.ascontiguousarray(pe[nm][:, f0:])
        in_maps.append(im)

    if "nc" not in _compiled:
        _compiled["nc"] = _build_nc()
    nc = _compiled["nc"]

    res = run_bass_kernel_spmd(nc, in_maps, core_ids=list(range(NCORES)))
    LAST_RESULTS = res

    # --- host postprocessing: stitch shipped boundary states ---
    S = [np.concatenate(
        [res.results[c]["ships"][:, i * BL:(i + 1) * BL].astype(np.float64)
         for c in range(NCORES)], axis=1) for i in range(NSHIP)]

    def ratio(a, b):
        return (a * b).sum(axis=0) / (b * b).sum(axis=0)

    sF = ratio(S[0], S[1]) * ratio(S[2], S[3]) * ratio(S[4], S[5]) \
        * ratio(S[6], S[7])                                      # [B]
    sB = ratio(S[9], S[10]) * ratio(S[11], S[12]) * ratio(S[13], S[14])
    Q = M64 @ S[15]                                              # [N,B]
    dot = (S[8] * Q).sum(axis=0)
    logZ = (np.log(dot) + np.log(sF) + np.log(sB) + D[ends]).sum()

    # gold score (f64, mirrors reference)
    tb = np.arange(B)
    emit_sc = np.take_along_axis(e64, target[:, :, None].astype(np.int64),
                                 axis=2)[..., 0]
    trans_sc = trans.astype(np.float64)[target[:-1], target[1:]]
    scores = emit_sc.copy()
    scores[1:] += trans_sc
    score = np.where(mask, scores, 0.0).sum()
    score += strans.astype(np.float64)[target[0]].sum()
    score += etrans.astype(np.float64)[target[ends, tb]].sum()

    loss = (logZ - score) / B
    return np.float32(loss)


# revision 37
# speedup vs baseline: 1.4794x; 1.0019x over previous
"""CRF NLL loss kernel for Trainium2 (8 NeuronCores, data-parallel over batch).

Math: the forward recurrence alpha_t = LSE_j(alpha_{t-1,j} + trans[j,k]) + emit_t
is computed in probability space:  P_t = Eemit_t * (Etrans^T @ P_{t-1})
with per-step normalizers d_t = mean_b LSE_k(emit[t,b,:]) (host-precomputed).

Parallel-segment decomposition: products of positive matrices contract in the
Hilbert projective metric (diagonal emission scalings are isometries), so a
chain started from an arbitrary positive vector converges to the true state's
DIRECTION within ~16 steps; magnitudes differ by one scalar per column which
the host recovers by stitching shipped boundary states.  The T-step serial
recurrence becomes 9 concurrent segments (5 forward + 4 backward), run as 3
TRIPLES of 76 lockstep steps each.  A triple advances with three matmuls into
thirds of one PSUM tile plus ONE combined [128,96] DVE multiply - the DVE's
per-instruction PSUM-access charge (125ns) is paid once per triple, fitting
3 triples under the ~680ns chain latency.  Interior segments use burn-ins of
15-31 steps (every spare step extends a burn-in, improving convergence).
Backward runs in X-space X_t = Ehat_t * (M @ X_{t+1}); variable sequence ends
are emission-rewrites on the host: padded steps hold the Perron vector r of
M = exp(trans) (scalar 1/lambda per step) and the single boundary step uses
v/r, v = M^{-1} exp(etrans), mapping r -> w exactly.
Host computes logZ_b = log(P(255) . M X(256)) + stitch scalars + D[end_b];
the gold-path score is pure gather work, done on host in f64.
"""

import numpy as np
import ml_dtypes

import concourse.bacc as bacc
import concourse.mybir as mybir
import concourse.tile as tile
from concourse.bass_utils import run_bass_kernel_spmd

T, B, N = 512, 256, 128
NCORES = 8
BL = B // NCORES          # 32 sequences per core
W3 = 3 * BL               # triple tile width
FIRST = 16                # emission steps folded into init DMAs
CHUNK = 15                # emission steps per DMA chunk (W3 wide)

BF = ml_dtypes.bfloat16

# members: (dir, consumed t-list, init, ships {step_idx: slot})
# slots: even = segment end state, odd = post-burn-in state; stitch pairs:
# fwd (0,1)(2,3)(4,5)(6,7) final 8; bwd (9,10)(11,12)(13,14) final 15.
def _mk_triples():
    F, Bk = "F", "B"
    return [
        dict(name="T1", phase=0, steps=61, period=647, members=[
            (F, list(range(1, 62)), {60: 0}),
            (F, list(range(46, 107)), {15: 1, 60: 2}),
            (F, list(range(91, 152)), {15: 3, 60: 4}),
        ]),
        dict(name="T2", phase=215, steps=69, period=745, members=[
            (F, list(range(135, 204)), {16: 5, 68: 6}),
            (F, list(range(187, 256)), {16: 7, 68: 8}),
            (Bk, list(range(510, 441, -1)), {68: 9}),
        ]),
        dict(name="T3", phase=430, steps=80, period=647, members=[
            (Bk, list(range(458, 378, -1)), {16: 10, 79: 11}),
            (Bk, list(range(395, 315, -1)), {16: 12, 79: 13}),
            (Bk, list(range(335, 255, -1)), {19: 14, 79: 15}),
        ]),
    ]

TRIPLES = _mk_triples()
NSHIP = 16

LAST_RESULTS = None       # BassKernelResults of the last run (for profiling)

_compiled = {}


def _build_nc():
    nc = bacc.Bacc("TRN2", target_bir_lowering=False, debug=False,
                   num_devices=NCORES)
    f32 = mybir.dt.float32
    bf16 = mybir.dt.bfloat16

    # init0 (HWDGE slot 1): [T1init | T2init | mF | mB | T1 first chunk]
    # init1 (SWDGE queue): [T3init | T2 first chunk | T3 first chunk]
    IW0 = 2 * W3 + 2 * N + FIRST * W3
    IW1 = W3 + 2 * FIRST * W3
    init0 = nc.dram_tensor("init0", [N, IW0], bf16, kind="ExternalInput")
    init1 = nc.dram_tensor("init1", [N, IW1], bf16, kind="ExternalInput")
    emd = {}
    for tr in TRIPLES:
        w = (tr["steps"] - FIRST) * W3
        emd[tr["name"]] = nc.dram_tensor("em" + tr["name"], [N, w], bf16,
                                         kind="ExternalInput")
    ships = nc.dram_tensor("ships", [N, NSHIP * BL], bf16,
                           kind="ExternalOutput")

    with tile.TileContext(nc) as tc:
        with (
            tc.tile_pool(name="const", bufs=1) as cpool,
            tc.tile_pool(name="emit", bufs=max(
                (tr["steps"] - FIRST + CHUNK - 1) // CHUNK
                for tr in TRIPLES)) as epool,
            tc.tile_pool(name="state", bufs=3) as spool,
            tc.tile_pool(name="ps", bufs=1, space="PSUM") as pspool,
        ):
            t00 = cpool.tile([N, IW0], bf16, tag="init0")
            nc.sync.dma_start(t00[:], init0[:])
            t01 = cpool.tile([N, IW1], bf16, tag="init1")
            nc.gpsimd.dma_start(t01[:], init1[:])

            mF = t00[:, 2 * W3:2 * W3 + N]
            mB = t00[:, 2 * W3 + N:2 * W3 + 2 * N]

            inits = {"T1": t00[:, 0:W3], "T2": t00[:, W3:2 * W3],
                     "T3": t01[:, 0:W3]}
            fbase = {"T1": (t00, 2 * W3 + 2 * N), "T2": (t01, W3),
                     "T3": (t01, W3 + FIRST * W3)}

            rt = {}
            chunk_reqs = []
            for tr in TRIPLES:
                nm = tr["name"]
                em_ap = []
                it, base = fbase[nm]
                for k in range(FIRST):
                    em_ap.append((it, base + k * W3))
                for c in range((tr["steps"] - FIRST + CHUNK - 1) // CHUNK):
                    lo = c * CHUNK * W3
                    hi = min(tr["steps"] - FIRST, (c + 1) * CHUNK) * W3
                    need = tr["phase"] + (FIRST + c * CHUNK) * tr["period"]
                    chunk_reqs.append((need, nm, lo, hi))
                # contiguous (direction, lo-member, n-members) matmul groups
                grp = []
                for mi, (dr, ts, sh) in enumerate(tr["members"]):
                    if grp and grp[-1][0] == dr:
                        grp[-1][2] += 1
                    else:
                        grp.append([dr, mi, 1])
                rt[nm] = dict(tr=tr, cur=inits[nm], em=em_ap, grp=grp)

            chunk_reqs.sort()
            for _, nm, lo, hi in chunk_reqs:
                ch_t = epool.tile([N, CHUNK * W3], bf16, tag="em" + nm)
                nc.sync.dma_start(ch_t[:, :hi - lo], emd[nm][:, lo:hi])
                for k in range((hi - lo) // W3):
                    rt[nm]["em"].append((ch_t, k * W3))

            events = []
            for tr in TRIPLES:
                for k in range(tr["steps"]):
                    events.append((tr["phase"] + k * tr["period"],
                                   tr["name"], k))
            events.sort()

            for _, nm, k in events:
                r = rt[nm]
                tr = r["tr"]
                ps_t = pspool.tile([N, W3], f32, tag="ps" + nm)
                cur = r["cur"]
                for dr, lo_m, n_m in r["grp"]:
                    nc.tensor.matmul(
                        ps_t[:, lo_m * BL:(lo_m + n_m) * BL],
                        mF if dr == "F" else mB,
                        cur[:, lo_m * BL:(lo_m + n_m) * BL],
                        start=True, stop=True)
                ch_t, off = r["em"][k]
                shd = [sh.get(k) for (_, _, sh) in tr["members"]]
                if any(s is not None for s in shd):
                    o_t = cpool.tile([N, W3], bf16, tag=f"sh{nm}_{k}")
                else:
                    o_t = spool.tile([N, W3], bf16, tag="s" + nm)
                nc.vector.tensor_tensor(o_t[:], ps_t[:],
                                        ch_t[:, off:off + W3],
                                        mybir.AluOpType.mult)
                r["cur"] = o_t
                for mi in range(3):
                    if shd[mi] is not None:
                        q = nc.sync if mi % 2 == 0 else nc.scalar
                        q.dma_start(
                            ships[:, shd[mi] * BL:(shd[mi] + 1) * BL],
                            o_t[:, mi * BL:(mi + 1) * BL])
    nc.compile()
    return nc


def kernel(emit, target, mask, trans, strans, etrans):
    global LAST_RESULTS
    emit = np.asarray(emit, dtype=np.float32)
    target = np.asarray(target, dtype=np.int32)
    mask = np.asarray(mask)
    trans = np.asarray(trans, dtype=np.float32)
    strans = np.asarray(strans, dtype=np.float32)
    etrans = np.asarray(etrans, dtype=np.float32)

    # --- host preprocessing ---
    e64 = emit.astype(np.float64)
    m_t = e64.max(axis=2, keepdims=True)
    lse = (m_t[..., 0] + np.log(np.exp(e64 - m_t).sum(axis=2)))  # [T,B]
    d = lse.mean(axis=1)
    d[0] = 0.0
    D = np.cumsum(d)

    eemn = np.exp(e64 - d[:, None, None])                        # [T,B,N]
    M64 = np.exp(trans.astype(np.float64))                       # [N,N] (j,k)
    w64 = np.exp(etrans.astype(np.float64))

    r = np.ones(N, dtype=np.float64)
    for _ in range(60):
        r = M64 @ r
        r /= r.mean()
    lam = float((r @ (M64 @ r)) / (r @ r))
    v = np.linalg.solve(M64, w64)

    L = mask.astype(np.int64).sum(axis=0)
    ends = L - 1

    P0 = np.exp(strans[None, :].astype(np.float64) + e64[0]).T   # [N,B]

    tt = np.arange(256, 511)
    EB = eemn[256:511].transpose(2, 0, 1).copy()                 # [N,255,B]
    pad = (tt[None, :] > L[:, None]).T[None, :, :]
    bnd = (tt[None, :] == L[:, None]).T[None, :, :]
    EB = np.where(pad, 1.0 / lam, EB)
    EB = np.where(bnd, (v / r)[:, None, None], EB)

    def em_at(t):
        if t <= 255:
            return eemn[t].T
        return EB[:, t - 256, :]

    X0 = np.empty((N, B), dtype=np.float64)
    full = L == T
    last = L == T - 1
    rest = ~(full | last)
    if full.any():
        X0[:, full] = (eemn[511, full, :] * w64[None, :]).T
    if last.any():
        X0[:, last] = v[:, None]
    if rest.any():
        X0[:, rest] = (r / lam)[:, None]

    # per-triple interleaved emissions [N, steps, 3, B]
    em_all = {}
    for tr in TRIPLES:
        ems = [np.stack([em_at(t) for t in ts], axis=1)
               for (_, ts, _) in tr["members"]]
        em_all[tr["name"]] = np.stack(ems, axis=2)               # [N,s,3,B]

    in_maps = []
    Mbf = M64.astype(BF)
    MTbf = np.ascontiguousarray(M64.T).astype(BF)
    for c in range(NCORES):
        sl = slice(c * BL, (c + 1) * BL)
        pe = {nm: em_all[nm][:, :, :, sl].reshape(N, -1).astype(BF)
              for nm in em_all}
        f0 = FIRST * W3
        ones_b = np.ones((N, BL), dtype=BF)
        t1i = np.concatenate([P0[:, sl].astype(BF), ones_b, ones_b], axis=1)
        t2i = np.concatenate([ones_b, ones_b, X0[:, sl].astype(BF)], axis=1)
        t3i = np.ones((N, 3 * BL), dtype=BF)
        im = {
            "init0": np.ascontiguousarray(np.concatenate(
                [t1i, t2i, Mbf, MTbf, pe["T1"][:, :f0]], axis=1)),
            "init1": np.ascontiguousarray(np.concatenate(
                [t3i, pe["T2"][:, :f0], pe["T3"][:, :f0]], axis=1)),
        }
        for nm in pe:
            im["em" + nm] = np.ascontiguousarray(pe[nm][:, f0:])
        in_maps.append(im)

    if "nc" not in _compiled:
        _compiled["nc"] = _build_nc()
    nc = _compiled["nc"]

    res = run_bass_kernel_spmd(nc, in_maps, core_ids=list(range(NCORES)))
    LAST_RESULTS = res

    # --- host postprocessing: stitch shipped boundary states ---
    S = [np.concatenate(
        [res.results[c]["ships"][:, i * BL:(i + 1) * BL].astype(np.float64)
         for c in range(NCORES)], axis=1) for i in range(NSHIP)]

    def ratio(a, b):
        return (a * b).sum(axis=0) / (b * b).sum(axis=0)

    sF = ratio(S[0], S[1]) * ratio(S[2], S[3]) * ratio(S[4], S[5]) \
        * ratio(S[6], S[7])                                      # [B]
    sB = ratio(S[9], S[10]) * ratio(S[11], S[12]) * ratio(S[13], S[14])
    Q = M64 @ S[15]                                              # [N,B]
    dot = (S[8] * Q).sum(axis=0)
    logZ = (np.log(dot) + np.log(sF) + np.log(sB) + D[ends]).sum()

    # gold score (f64, mirrors reference)
    tb = np.arange(B)
    emit_sc = np.take_along_axis(e64, target[:, :, None].astype(np.int64),
                                 axis=2)[..., 0]
    trans_sc = trans.astype(np.float64)[target[:-1], target[1:]]
    scores = emit_sc.copy()
    scores[1:] += trans_sc
    score = np.where(mask, scores, 0.0).sum()
    score += strans.astype(np.float64)[target[0]].sum()
    score += etrans.astype(np.float64)[target[ends, tb]].sum()

    loss = (logZ - score) / B
    return np.float32(loss)
